# revision 2
# baseline (speedup 1.0000x reference)
"""TRN2 Bass kernel for the ESN (echo-state-network) recurrence:

    U   = inputs @ W_in + b_in                              # [B, T, N]
    x0  = 0.5 * tanh(U[:, 0])
    x_t = 0.5*x_{t-1} + 0.5*tanh(U[:, t] + x_{t-1} @ W_res + b_res)
    X   = stack([x0 ... x_{T-1}], 1)                        # [B, T, N]

Sharding: data-parallel over batch B=128 -> 16 per NeuronCore x 8 cores
(the scan recurrence is independent per batch element; weights are
replicated; no cross-core communication).

Per-core design:
  - State kept in transposed packed layout  x_tile[p, c*16+b] =
    x_t[b, c*128+p]  (c = N-chunk 0..7), so the per-step matmul
    z^T = W_res^T-blocks @ x^T runs with W_res blocks as the PE
    stationary operand ([128,128] lhsT tiles, natural W_res layout)
    and the state as the 16-wide moving operand.  All elementwise work
    (tanh on ScalarE, leak blend on VectorE) runs on full 128
    partitions.
  - W_in + (b_in + b_res) are folded in as a 9th contraction chunk
    (K=65: 64 input dims + a ones-row scaled by the bias), so the
    pre-activation lands fully accumulated in PSUM; tanh needs no
    extra adds.
  - lhsT/rhs in fp16 (PSUM accumulation stays fp32): single-pass
    matmul + FastWeightLoad (fp32 matmuls lower to 2 HI/LO passes and
    load weights at half rate).
  - Output written per step to a DRAM staging tensor in the packed
    layout; the host (this function) does the final layout transpose
    during the gather/unshard step.
"""

import sys

sys.path.insert(0, "/opt/trn_rl_repo")

from contextlib import ExitStack

import numpy as np

try:  # persistent jit cache so repeated runs skip the ~3 min walrus compile
    import jax

    jax.config.update("jax_compilation_cache_dir", "/var/tmp/jax_comp_cache")
    jax.config.update("jax_persistent_cache_min_compile_time_secs", 0.0)
    jax.config.update("jax_persistent_cache_min_entry_size_bytes", 0)
except Exception:
    pass

import concourse.bass as bass
import concourse.tile as tile
from concourse import bacc, mybir
from concourse.bass_utils import run_bass_kernel_spmd

F32 = mybir.dt.float32
F16 = mybir.dt.float16

N_CORES = 8
B = 128
B_LOC = 16  # batches per core
T = 512
D = 64
N = 1024
NC = 8  # N chunks of 128
P = 128
TANH = mybir.ActivationFunctionType.Tanh
ALU = mybir.AluOpType


def build_kernel(t_steps=T, w_dtype=F16):
    nc = bacc.Bacc(None, target_bir_lowering=False)
    inputs = nc.dram_tensor("inputs", [B_LOC, t_steps, D], F32, kind="ExternalInput")
    W_in = nc.dram_tensor("W_in", [D, N], F32, kind="ExternalInput")
    b_in = nc.dram_tensor("b_in", [N], F32, kind="ExternalInput")
    W_res = nc.dram_tensor("W_res", [N, N], F32, kind="ExternalInput")
    b_res = nc.dram_tensor("b_res", [N], F32, kind="ExternalInput")
    # Staging output: Xs[t, p, c*16+b] = x_t[b, c*128+p] (host rearranges).
    x_dt = F32 if w_dtype == F32 else w_dtype
    Xs = nc.dram_tensor("Xs", [t_steps, P, P], x_dt, kind="ExternalOutput")

    with tile.TileContext(nc) as tc, ExitStack() as ctx:
        consts = ctx.enter_context(tc.tile_pool(name="consts", bufs=1))
        state = ctx.enter_context(tc.tile_pool(name="state", bufs=3))
        psum = ctx.enter_context(
            tc.tile_pool(name="psum", bufs=4, space=bass.MemorySpace.PSUM)
        )

        # ---- constants ----
        # W_res lhsT tiles: wt[p, c, c', m] = W_res[c*128+p, c'*128+m]
        wt = consts.tile([P, NC, NC, P], w_dtype, tag="wt")
        w_src = W_res[:].rearrange("(c p) (q m) -> p c q m", p=P, m=P)
        if w_dtype == F32:
            nc.gpsimd.dma_start(out=wt, in_=w_src)
        else:
            wt32 = consts.tile([P, NC, NC, P], F32, tag="wt32")
            nc.gpsimd.dma_start(out=wt32, in_=w_src)
            nc.vector.tensor_copy(
                out=wt.rearrange("p c q m -> p (c q m)"),
                in_=wt32.rearrange("p c q m -> p (c q m)"),
            )

        # chunk-9 lhsT: rows 0..63 = W_in columns, row 64 = bias.
        # wi row64 = b_in + b_res (steps >= 1); wi0 row64 = b_in (step 0).
        wi32 = consts.tile([D + 1, NC, P], F32, tag="wi32")
        wi032 = consts.tile([D + 1, NC, P], F32, tag="wi032")
        nc.gpsimd.dma_start(
            out=wi32[0:D], in_=W_in[:].rearrange("d (q m) -> d q m", m=P)
        )
        nc.gpsimd.dma_start(
            out=wi032[0:D], in_=W_in[:].rearrange("d (q m) -> d q m", m=P)
        )
        nc.gpsimd.dma_start(
            out=wi032[D : D + 1], in_=b_in[:].rearrange("(z q m) -> z q m", z=1, m=P)
        )
        bres_row = consts.tile([D + 1, NC, P], F32, tag="bres")
        nc.gpsimd.dma_start(
            out=bres_row[D : D + 1],
            in_=b_res[:].rearrange("(z q m) -> z q m", z=1, m=P),
        )
        nc.vector.tensor_tensor(
            out=wi32[D : D + 1].rearrange("z q m -> z (q m)"),
            in0=wi032[D : D + 1].rearrange("z q m -> z (q m)"),
            in1=bres_row[D : D + 1].rearrange("z q m -> z (q m)"),
            op=ALU.add,
        )
        if w_dtype == F32:
            wi, wi0 = wi32, wi032
        else:
            wi = consts.tile([D + 1, NC, P], w_dtype, tag="wi")
            wi0 = consts.tile([D + 1, NC, P], w_dtype, tag="wi0")
            nc.vector.tensor_copy(
                out=wi.rearrange("d q m -> d (q m)"),
                in_=wi32.rearrange("d q m -> d (q m)"),
            )
            nc.vector.tensor_copy(
                out=wi0.rearrange("d q m -> d (q m)"),
                in_=wi032.rearrange("d q m -> d (q m)"),
            )

        # inputs transposed: inp_sb[d, b, t] = inputs[b, t, d]; row 64 = ones
        inp32 = consts.tile([D + 1, B_LOC, t_steps], F32, tag="inp32")
        nc.sync.dma_start_transpose(
            out=inp32[0:D].rearrange("d b t -> d (b t)"),
            in_=inputs[:].rearrange("b t d -> (b t) d"),
        )
        nc.vector.memset(inp32[D : D + 1].rearrange("d b t -> d (b t)"), 1.0)
        if w_dtype == F32:
            inp_sb = inp32
        else:
            inp_sb = consts.tile([D + 1, B_LOC, t_steps], w_dtype, tag="inp")
            nc.vector.tensor_copy(
                out=inp_sb.rearrange("d b t -> d (b t)"),
                in_=inp32.rearrange("d b t -> d (b t)"),
            )

        xs_view = Xs[:]  # [T, P, P]
        H = NC // 2  # c' chunks per half
        HB = H * B_LOC  # 64 cols per half

        def half_step(t, h, rhs_x, wi_t):
            ps = psum.tile([P, HB], F32, tag="ps")
            for j in range(H):
                cp = h * H + j
                out = ps[:, j * B_LOC : (j + 1) * B_LOC]
                rhs_u = inp_sb[:, :, t : t + 1]
                nc.tensor.matmul(
                    out, wi_t[:, cp, :], rhs_u, start=True, stop=rhs_x is None
                )
                if rhs_x is not None:
                    for c in range(NC):
                        xsrc = rhs_x[c // H]
                        rhs = xsrc[:, (c % H) * B_LOC : (c % H + 1) * B_LOC]
                        nc.tensor.matmul(
                            out, wt[:, c, cp, :], rhs, start=False, stop=(c == NC - 1)
                        )
            return ps

        def half_post(t, h, ps, xh_prev_h):
            th = state.tile([P, HB], F32, tag=f"th{h}")
            nc.scalar.activation(out=th, in_=ps, func=TANH)
            xn = state.tile([P, HB], x_dt, tag=f"x{h}")
            if xh_prev_h is None:
                nc.vector.tensor_scalar_mul(xn, th, 0.5)  # x0 = 0.5*tanh(u0)
            else:
                # x_t = 0.5*tanh + xh_{t-1}   (xh = x/2)
                nc.vector.scalar_tensor_tensor(
                    out=xn, in0=th, scalar=0.5, in1=xh_prev_h, op0=ALU.mult, op1=ALU.add
                )
            xh = state.tile([P, HB], x_dt, tag=f"xh{h}")
            nc.vector.tensor_scalar_mul(xh, xn, 0.5)
            nc.sync.dma_start(out=xs_view[t, :, h * HB : (h + 1) * HB], in_=xn)
            return xn, xh

        ps0 = half_step(0, 0, None, wi0)
        ps1 = half_step(0, 1, None, wi0)
        xa, xha = half_post(0, 0, ps0, None)
        xb, xhb = half_post(0, 1, ps1, None)
        for t in range(1, t_steps):
            ps0 = half_step(t, 0, (xa, xb), wi)
            ps1 = half_step(t, 1, (xa, xb), wi)
            xa_n, xha_n = half_post(t, 0, ps0, xha)
            xb_n, xhb_n = half_post(t, 1, ps1, xhb)
            xa, xb, xha, xhb = xa_n, xb_n, xha_n, xhb_n

    nc.compile()
    return nc


def unstage(Xs):
    """Xs [T,128,128] with Xs[t, p, c*16+b] = x_t[b, c*128+p] -> [16, T, N]."""
    t_steps = Xs.shape[0]
    v = Xs.astype(np.float32).reshape(t_steps, P, NC, B_LOC)
    return np.ascontiguousarray(v.transpose(3, 0, 2, 1)).reshape(B_LOC, t_steps, N)


_NC_CACHE = {}


def _get_nc(t_steps, w_dtype=F16):
    key = (t_steps, w_dtype)
    if key not in _NC_CACHE:
        _NC_CACHE[key] = build_kernel(t_steps, w_dtype)
    return _NC_CACHE[key]


def run_sharded(inputs, W_in, b_in, W_res, b_res, trace=False, w_dtype=F16):
    """Run the SPMD kernel on 8 cores; returns (X_full, BassKernelResults)."""
    b_total, t_steps, _ = inputs.shape
    assert b_total == B and t_steps == T
    nc = _get_nc(t_steps, w_dtype)
    shared = {
        "W_in": np.ascontiguousarray(W_in, np.float32),
        "b_in": np.ascontiguousarray(b_in, np.float32),
        "W_res": np.ascontiguousarray(W_res, np.float32),
        "b_res": np.ascontiguousarray(b_res, np.float32),
    }
    in_maps = [
        {
            "inputs": np.ascontiguousarray(
                inputs[c * B_LOC : (c + 1) * B_LOC], np.float32
            ),
            **shared,
        }
        for c in range(N_CORES)
    ]
    res = run_bass_kernel_spmd(
        nc, in_maps, core_ids=list(range(N_CORES)), trace=trace
    )
    X = np.concatenate([unstage(r["Xs"]) for r in res.results], axis=0)
    return X, res


def kernel(**inputs):
    X, _ = run_sharded(
        inputs["inputs"],
        inputs["W_in"],
        inputs["b_in"],
        inputs["W_res"],
        inputs["b_res"],
    )
    return X.astype(np.float32)
